# revision 8
# baseline (speedup 1.0000x reference)
"""BinaryLinear Trainium2 kernel: out = sign(x) @ sign(W).T

x: (4, 4096, 1024) f32, W: (1024, 1024) f32 -> out (4, 4096, 1024) f32.

Strategy (8 NeuronCores, data-parallel over flattened batch*seq):
  - Each core gets a [2048, 1024] row-shard of x and the full W.
  - x is re-laid-out on the host (pure permutation, no arithmetic) so the
    contraction index i lands on SBUF partitions directly: per core the DRAM
    tensor is [8 chunks * 128 p, (4 j, 2 c, 2 t, 128 u)] f32 with
    i = 256 j + 128 c + p and row m = 256 ch + 2 u + t. This removes the
    on-chip transpose entirely and loads with 8 KiB-per-partition contiguous
    descriptors.
  - Per chunk (256 rows): DMA 1 MiB -> ACT Sign (f32 -> fp8e4, +-1/0 exact)
    -> 16 fp8 DoubleRow matmuls (K=256 each) accumulating [128 m, 512 o]
    PSUM tiles -> DVE copy PSUM -> SBUF as float16 -> 0.5 MiB DMA out.
  - Outputs are exact integers |v| <= 1024, representable exactly in fp16,
    so stores are half-width; the host upcasts to f32. The evens/odds row
    interleave (t bit) makes each store descriptor cover 2 adjacent DRAM
    rows = 4 KiB.
  - W is repacked once on the host: wq[p, (j, c, o)] = sign(W)[o, i] fp8;
    each core DMAs the packed 1 MiB tensor once.

All arithmetic is exact: sign values are +-1/0 (exact in fp8e4), the PE
accumulates in fp32, and |out| <= 1024 is exact in fp16.
"""

import numpy as np

P = 128
K = 1024  # in_features
N = 1024  # out_features
N_CORES = 8
M_TOTAL = 4 * 4096
M_PER_CORE = M_TOTAL // N_CORES
MC = 256  # rows per chunk
N_CH = M_PER_CORE // MC


def build_binary_linear(tc, out, x, w):
    """Emit the per-core Tile kernel.

    out: DRAM [M_PER_CORE, N] f16, x: DRAM [N_CH*P, 8*MC] f32 (host-packed),
    w: DRAM [P, 8*N] fp8 (host-packed).
    """
    import concourse.mybir as mybir

    nc = tc.nc
    f32 = mybir.dt.float32
    f16 = mybir.dt.float16
    fp8 = mybir.dt.float8e4
    Sign = mybir.ActivationFunctionType.Sign
    DR = mybir.MatmulPerfMode.DoubleRow

    with (
        tc.tile_pool(name="wsb", bufs=1) as wpool,
        tc.tile_pool(name="xin", bufs=4) as xin_pool,
        tc.tile_pool(name="x8p", bufs=3) as x8_pool,
        tc.tile_pool(name="osb", bufs=4) as out_pool,
        tc.tile_pool(name="ps", bufs=2, space="PSUM") as psum_pool,
    ):
        # ---- W: host-packed fp8 [128, 8*1024]; wq[p, (j, c, o)]
        # = sign(W)[o, i] with i = 256j + 128c + p. One 1MB DMA, triggered
        # from the DVE queue so it doesn't delay the x-load chain on Pool.
        wT = wpool.tile([P, 8 * N], fp8, name="wT")
        nc.sync.dma_start(out=wT, in_=w)
        w4 = wT.rearrange("p (j c o) -> p j c o", j=4, c=2)

        for ch in range(N_CH):
            xf = xin_pool.tile([P, 8 * MC], f32, tag="xf", name=f"xf{ch}")
            nc.gpsimd.dma_start(out=xf, in_=x[ch * P : (ch + 1) * P, :])
            x8 = x8_pool.tile([P, 8 * MC], fp8, tag="x8", name=f"x8{ch}")
            nc.scalar.activation(out=x8, in_=xf, func=Sign)
            x84 = x8.rearrange("p (j c m) -> p j c m", j=4, c=2)

            osb = out_pool.tile([P, 2 * N], f16, tag="osb", name=f"osb{ch}")
            osb2 = osb.rearrange("p (b o) -> p b o", b=2)
            for t in range(MC // P):
                ps = [
                    psum_pool.tile([P, 512], f32, tag=f"ps{t}{h}", name=f"ps{t}{h}")
                    for h in range(2)
                ]
                for j in range(4):
                    lhsT = x84[:, j, :, t * P : (t + 1) * P]
                    for h in range(2):
                        nc.tensor.matmul(
                            ps[h],
                            lhsT=lhsT,
                            rhs=w4[:, j, :, h * 512 : (h + 1) * 512],
                            start=(j == 0),
                            stop=(j == 3),
                            perf_mode=DR,
                        )
                for h in range(2):
                    nc.vector.tensor_copy(
                        out=osb2[:, t, h * 512 : (h + 1) * 512], in_=ps[h]
                    )
            # out rows ch*256 + 2p + t  <-  osb[p, t, o]; triggered from the
            # SP queue so it never blocks a load trigger on Pool.
            nc.sync.dma_start(
                out=out[ch * MC : (ch + 1) * MC].rearrange("(p b) o -> p (b o)", b=2),
                in_=osb,
            )


def _rewire_waits(nc, n_x_bufs, n_osb_bufs):
    """Replace Tile's conservative / lane-aliased DMA waits with exact
    producer-based waits computed from the scheduled stream.

      xf_load[0]   <- (nothing)
      xf_load[ch]  <- xf_load[ch-1] completion (chains the load DMAs so the
                      first chunk finishes at full bandwidth instead of
                      fair-sharing with all prefetched loads), and
                      sign[ch - n_x_bufs] (xf-slot WAR)
      w_load       <- (nothing; first on the DVE queue)
      sign[ch]     <- xf_load[ch] completion (RAW) + keep Tile's PE wait
                      (x8-slot WAR)
      copy[ch][0]  <- store[ch - n_osb_bufs] completion (osb-slot WAR)
                      + keep Tile's PE wait (psum RAW)
      copy[ch][k]  <- keep Tile's PE wait only
      store[ch]    <- copy[ch][last] completion (RAW)

    Waits are emitted as (producer's update-sem >= cumulative value after
    it). Each referenced sem has a single updater in this kernel, so
    cumulative values are unambiguous.
    """
    import concourse.mybir as mybir

    insts = []
    for f in nc.m.functions:
        for bb in f.blocks:
            insts.extend(bb.instructions)

    cum = {}
    upd_after = {}  # inst name -> (sem_name, sem_id, cum_value_after)
    lane_order = {}  # inst name -> SyncWait enforcing same-lane completion order
    seqs = {k: [] for k in ("wT", "xf", "x8", "osb", "out")}
    for ins in insts:
        si = getattr(ins, "sync_info", None)
        if si is None:
            continue
        for u in si.on_update or []:
            prev = cum.get(u.ant_name, 0)
            if prev > 0 and (
                u.ant_name.startswith("DMAHW") or u.ant_name.startswith("DMASW")
            ):
                lane_order[ins.name] = mybir.SyncWait(
                    sync_type="semaphore",
                    id=u.id,
                    ant_name=u.ant_name,
                    wait_mode="sem-ge-imm",
                    wait_value=prev,
                )
            cum[u.ant_name] = prev + u.update_value
            upd_after[ins.name] = (u.ant_name, u.id, cum[u.ant_name])
        memref = str(getattr(ins.outs[0], "memref", "")) if ins.outs else ""
        tn = type(ins).__name__
        for pref, want_tn in (
            ("wT", "InstDMACopy"),
            ("xf", "InstDMACopy"),
            ("x8", "InstActivation"),
            ("osb", "InstTensorCopy"),
            ("out", "InstDMACopy"),
        ):
            if tn == want_tn and memref.startswith(pref):
                seqs[pref].append(ins)
                break

    def wait_on(producer_ins):
        sem_name, sem_id, v = upd_after[producer_ins.name]
        return mybir.SyncWait(
            sync_type="semaphore",
            id=sem_id,
            ant_name=sem_name,
            wait_mode="sem-ge-imm",
            wait_value=v,
        )

    def keep_engine_waits(ins):
        return [
            w
            for w in (ins.sync_info.on_wait or [])
            if not (
                w.ant_name.startswith("DMAHW")
                or w.ant_name.startswith("DMASW")
                or w.ant_name.startswith("Activation")
                or w.ant_name.startswith("DVE")
            )
        ]

    def set_waits(ins, producers, extra=()):
        si = ins.sync_info
        waits = [wait_on(p) for p in producers if p is not None] + list(extra)
        lo = lane_order.get(ins.name)
        if lo is not None:
            waits.append(lo)
        ins.sync_info = mybir.SyncInfo(
            on_wait=waits, on_update=list(si.on_update or [])
        )

    n_ch = len(seqs["xf"])
    n_cp = len(seqs["osb"]) // max(len(seqs["out"]), 1)
    for ch, ins in enumerate(seqs["xf"]):
        deps = []
        if ch > 0:
            deps.append(seqs["xf"][ch - 1])
        if ch >= n_x_bufs:
            deps.append(seqs["x8"][ch - n_x_bufs])
        set_waits(ins, deps)
    for ins in seqs["wT"]:
        set_waits(ins, [])
    for ch, ins in enumerate(seqs["x8"]):
        set_waits(ins, [seqs["xf"][ch]], extra=keep_engine_waits(ins))
    for i, ins in enumerate(seqs["osb"]):
        ch, k = divmod(i, n_cp)
        deps = []
        if k == 0 and ch >= n_osb_bufs:
            deps.append(seqs["out"][ch - n_osb_bufs])
        set_waits(ins, deps, extra=keep_engine_waits(ins))
    for ch, ins in enumerate(seqs["out"]):
        set_waits(ins, [seqs["osb"][(ch + 1) * n_cp - 1]])
    return {k: len(v) for k, v in seqs.items()}


def _legalize_dma_waits(nc):
    """Walrus caps in-struct sem waits (DMA_DIRECT2D takes 1, DMACopy 2).

    Tile's sem assignment is not transitively minimal and can emit 2-4 waits
    on DMA instructions. Hoist the excess into InstEventSemaphore wait-only
    instructions inserted just before the DMA on its triggering queue. This
    is sound: the queue executes the hoisted wait strictly before pushing the
    DMA descriptor, so the dependency is enforced (more conservatively) at
    trigger time instead of ring-pop time.
    """
    import concourse.mybir as mybir

    limits = {
        "InstDmaTransposeAnt": 1,
        "InstDMACopy": 1,
        "InstTensorCopy": 1,
        "InstActivation": 1,
        "InstMatmult": 1,
        "InstLdweights": 1,
        "InstMemset": 1,
        "InstTensorTensor": 1,
        "InstDrain": 1,
    }
    n_hoisted = 0
    for f in nc.m.functions:
        for bb in f.blocks:
            new_list = []
            for ins in bb.instructions:
                lim = limits.get(type(ins).__name__)
                si = getattr(ins, "sync_info", None)
                waits = list(si.on_wait) if si is not None and si.on_wait else []
                if lim is not None and len(waits) > lim:
                    # keep data-producer (engine-sem) waits in-struct first,
                    # then the freshest DMA-lane waits; hoist the rest
                    def keep_rank(w):
                        is_lane = w.ant_name.startswith(
                            "DMAHW"
                        ) or w.ant_name.startswith("DMASW")
                        return (1 if is_lane else 0, -w.wait_value)

                    waits_sorted = sorted(waits, key=keep_rank)
                    keep, hoist = waits_sorted[:lim], waits_sorted[lim:]
                    for ci in range(0, len(hoist), 2):
                        chunk = hoist[ci : ci + 2]
                        ev = mybir.InstEventSemaphore(
                            name=f"{ins.name}-prewait{ci // 2}",
                            engine=ins.engine,
                            ins=[],
                            outs=[],
                            sync_info=mybir.SyncInfo(on_wait=chunk, on_update=[]),
                        )
                        nc.inst_map[ev.name] = ev
                        new_list.append(ev)
                        n_hoisted += len(chunk)
                    ins.sync_info = mybir.SyncInfo(
                        on_wait=keep, on_update=list(si.on_update or [])
                    )
                new_list.append(ins)
            bb.instructions[:] = new_list
    return n_hoisted


def _build_nc():
    import concourse.bass as bass
    import concourse.mybir as mybir
    from concourse import tile

    nc = bass.Bass("TRN2", target_bir_lowering=False, num_swdge_queues=4)
    x_d = nc.dram_tensor(
        "x", [N_CH * P, 8 * MC], mybir.dt.float32, kind="ExternalInput"
    )
    w_d = nc.dram_tensor("W", [P, 8 * N], mybir.dt.float8e4, kind="ExternalInput")
    out_d = nc.dram_tensor(
        "out", [M_PER_CORE, N], mybir.dt.float16, kind="ExternalOutput"
    )
    with tile.TileContext(nc) as tc:
        build_binary_linear(tc, out_d.ap(), x_d.ap(), w_d.ap())
    counts = _rewire_waits(nc, n_x_bufs=4, n_osb_bufs=4)
    assert counts == {"wT": 1, "xf": N_CH, "x8": N_CH, "osb": 4 * N_CH, "out": N_CH}, counts
    _legalize_dma_waits(nc)
    return nc


_cached = {}


def _get_nc():
    if "nc" not in _cached:
        _cached["nc"] = _build_nc()
    return _cached["nc"]


def kernel(x, W, _trace=False):
    from concourse import bass_utils

    import ml_dtypes

    xf = np.asarray(x, dtype=np.float32).reshape(M_TOTAL, K)
    # host re-layout (pure permutation): per core [ (ch, p), (j, c, t, u) ]
    # with m = 2048*core + 256*ch + 2u + t and i = 256j + 128c + p
    T = xf.reshape(N_CORES, N_CH, P, 2, 4, 2, P)  # (core, ch, u, t, j, c, p)
    xh = np.ascontiguousarray(T.transpose(0, 1, 6, 4, 5, 3, 2)).reshape(
        N_CORES, N_CH * P, 8 * MC
    )
    # pack sign(W) fp8: wq[p, (j, c, o)] = sign(W)[o, 256j + 128c + p]
    sT = np.sign(np.asarray(W, dtype=np.float32)).T.astype(ml_dtypes.float8_e4m3)
    wq = np.ascontiguousarray(
        sT.reshape(4, 2, P, N).transpose(2, 0, 1, 3)
    ).reshape(P, 8 * N)
    in_maps = [{"x": xh[i], "W": wq} for i in range(N_CORES)]
    nc = _get_nc()
    res = bass_utils.run_bass_kernel_spmd(
        nc, in_maps, core_ids=list(range(N_CORES)), trace=_trace
    )
    out = np.concatenate([r["out"] for r in res.results], axis=0)
    out = out.astype(np.float32).reshape(4, 4096, N)
    if _trace:
        kernel.last_results = res
    return out


# revision 10
# speedup vs baseline: 1.0772x; 1.0772x over previous
"""BinaryLinear Trainium2 kernel: out = sign(x) @ sign(W).T

x: (4, 4096, 1024) f32, W: (1024, 1024) f32 -> out (4, 4096, 1024) f32.

Strategy (8 NeuronCores, data-parallel over flattened batch*seq):
  - Each core gets a [2048, 1024] row-shard of x and the full W.
  - x is re-laid-out on the host (pure permutation, no arithmetic) so the
    contraction index i lands on SBUF partitions directly: per core the DRAM
    tensor is [8 chunks * 128 p, (4 j, 2 c, 2 t', 128 u)] f32 with
    i = 256 j + 128 c + p and row m = 512 g + 4 u + 2 b0 + t' for chunk
    ch = 2 g + b0. This removes the on-chip transpose entirely and loads
    with 8 KiB-per-partition contiguous descriptors.
  - A single DMA instruction is capped at ~110 GB/s by descriptor
    generation, so each 1 MiB chunk load is split into 4 sub-DMAs
    (partition quarters) that run concurrently (~420 GB/s aggregate), and
    each sub-DMA chains on the same sub-lane of the previous chunk so
    exactly 4 are in flight and chunks complete in order.
  - Per chunk (256 rows): ACT Sign (f32 -> fp8e4, +-1/0 exact) -> 16 fp8
    DoubleRow matmuls (K=256 each) accumulating [128 m, 512 o] PSUM tiles
    -> DVE copy PSUM -> SBUF as float16.
  - Outputs are exact integers |v| <= 1024, representable exactly in fp16,
    so stores are half-width; the host upcasts to f32. The 4-way row
    interleave (m = 512g + 4u + b) makes each partition hold 4 adjacent
    DRAM rows, i.e. 8 KiB store descriptors. Stores cover one 512-row
    group (two chunks) and are split into 2 sub-DMAs on the SP (sync)
    queue so they never block a load trigger on the Pool queue.
  - W is repacked once on the host: wq[p, (j, c, o)] = sign(W)[o, i] fp8;
    each core DMAs the packed 1 MiB tensor once (SP queue, concurrent
    with the first x chunk).

All arithmetic is exact: sign values are +-1/0 (exact in fp8e4), the PE
accumulates in fp32, and |out| <= 1024 is exact in fp16.
"""

import numpy as np

P = 128
K = 1024  # in_features
N = 1024  # out_features
N_CORES = 8
M_TOTAL = 4 * 4096
M_PER_CORE = M_TOTAL // N_CORES
MC = 256  # rows per chunk
N_CH = M_PER_CORE // MC
N_SUB = 4  # load sub-DMAs per chunk
N_GRP = N_CH // 2  # 512-row store groups


def build_binary_linear(tc, out, x, w):
    """Emit the per-core Tile kernel.

    out: DRAM [M_PER_CORE, N] f16, x: DRAM [N_CH*P, 8*MC] f32 (host-packed),
    w: DRAM [P, 8*N] fp8 (host-packed).
    """
    import concourse.mybir as mybir

    nc = tc.nc
    f32 = mybir.dt.float32
    f16 = mybir.dt.float16
    fp8 = mybir.dt.float8e4
    Sign = mybir.ActivationFunctionType.Sign
    DR = mybir.MatmulPerfMode.DoubleRow
    PQ = P // N_SUB

    with (
        tc.tile_pool(name="wsb", bufs=1) as wpool,
        tc.tile_pool(name="xin", bufs=4) as xin_pool,
        tc.tile_pool(name="x8p", bufs=3) as x8_pool,
        tc.tile_pool(name="osb", bufs=4) as out_pool,
        tc.tile_pool(name="ps", bufs=2, space="PSUM") as psum_pool,
    ):
        # ---- W: host-packed fp8 [128, 8*1024]; wq[p, (j, c, o)]
        # = sign(W)[o, i] with i = 256j + 128c + p. One 1MB DMA on SP. ----
        wT = wpool.tile([P, 8 * N], fp8, name="wT")
        nc.sync.dma_start(out=wT, in_=w)
        w4 = wT.rearrange("p (j c o) -> p j c o", j=4, c=2)

        osbs = {}
        for ch in range(N_CH):
            g, b0 = divmod(ch, 2)
            xf = xin_pool.tile([P, 8 * MC], f32, tag="xf", name=f"xf{ch}")
            for q in range(N_SUB):
                nc.gpsimd.dma_start(
                    out=xf[PQ * q : PQ * (q + 1), :],
                    in_=x[ch * P + PQ * q : ch * P + PQ * (q + 1), :],
                )
            x8 = x8_pool.tile([P, 8 * MC], fp8, tag="x8", name=f"x8{ch}")
            nc.scalar.activation(out=x8, in_=xf, func=Sign)
            x84 = x8.rearrange("p (j c m) -> p j c m", j=4, c=2)

            if b0 == 0:
                osbs[g] = out_pool.tile([P, 4 * N], f16, tag="osb", name=f"osb{g}")
            osb2 = osbs[g].rearrange("p (b o) -> p b o", b=4)
            for t in range(MC // P):
                ps = [
                    psum_pool.tile([P, 512], f32, tag=f"ps{t}{h}", name=f"ps{t}{h}")
                    for h in range(2)
                ]
                for j in range(4):
                    lhsT = x84[:, j, :, t * P : (t + 1) * P]
                    for h in range(2):
                        nc.tensor.matmul(
                            ps[h],
                            lhsT=lhsT,
                            rhs=w4[:, j, :, h * 512 : (h + 1) * 512],
                            start=(j == 0),
                            stop=(j == 3),
                            perf_mode=DR,
                        )
                for h in range(2):
                    nc.vector.tensor_copy(
                        out=osb2[:, 2 * b0 + t, h * 512 : (h + 1) * 512], in_=ps[h]
                    )
            if b0 == 1:
                # out rows 512g + 4p + b  <-  osb[p, b, o]; two sub-DMAs on
                # the SP queue (one DMA instruction is descgen-limited).
                for q in range(2):
                    r0 = 512 * g + 256 * q
                    nc.sync.dma_start(
                        out=out[r0 : r0 + 256].rearrange("(p b) o -> p (b o)", b=4),
                        in_=osbs[g][64 * q : 64 * (q + 1), :],
                    )


def _rewire_waits(nc, n_x_bufs):
    """Replace Tile's conservative / lane-aliased DMA waits with exact
    producer-based waits computed from the scheduled stream.

      xf[ch][k]  <- xf[ch-1][k] completion (per-sublane chain: exactly 4
                    sub-DMAs in flight, chunks complete in order at full
                    aggregate bandwidth), and sign[ch - n_x_bufs]
                    (xf-slot WAR)
      w_load     <- (nothing; first on the SP queue)
      sign[ch]   <- all 4 xf[ch] sub completions (RAW) + keep Tile's PE
                    wait (x8-slot WAR)
      copy[...]  <- keep Tile's PE wait only (psum RAW; osb pool has one
                    buffer per group, no WAR)
      store[g,q] <- last copy of group g (RAW)

    Waits are emitted as (producer's update-sem >= cumulative value after
    it); lane-order waits keep same-sem DMA updates ordered.
    """
    import concourse.mybir as mybir

    insts = []
    for f in nc.m.functions:
        for bb in f.blocks:
            insts.extend(bb.instructions)

    cum = {}
    upd_after = {}  # inst name -> (sem_name, sem_id, cum_value_after)
    lane_order = {}  # inst name -> SyncWait enforcing same-lane completion order
    xf_subs = {}  # ch -> [inst]
    signs = {}  # ch -> inst
    copies = {}  # g -> [inst]
    stores = {}  # g -> [inst]
    w_loads = []
    for ins in insts:
        si = getattr(ins, "sync_info", None)
        if si is None:
            continue
        for u in si.on_update or []:
            prev = cum.get(u.ant_name, 0)
            if prev > 0 and (
                u.ant_name.startswith("DMAHW") or u.ant_name.startswith("DMASW")
            ):
                lane_order[ins.name] = mybir.SyncWait(
                    sync_type="semaphore",
                    id=u.id,
                    ant_name=u.ant_name,
                    wait_mode="sem-ge-imm",
                    wait_value=prev,
                )
            cum[u.ant_name] = prev + u.update_value
            upd_after[ins.name] = (u.ant_name, u.id, cum[u.ant_name])
        memref = str(getattr(ins.outs[0], "memref", "")) if ins.outs else ""
        tn = type(ins).__name__
        if tn == "InstDMACopy" and memref.startswith("xf"):
            ch = int(memref[2 : memref.index("_")])
            xf_subs.setdefault(ch, []).append(ins)
        elif tn == "InstDMACopy" and memref.startswith("wT"):
            w_loads.append(ins)
        elif tn == "InstDMACopy" and memref.startswith("out"):
            off = int(ins.outs[0].offset)  # in f16 elements
            g = off // (512 * N)
            stores.setdefault(g, []).append(ins)
        elif tn == "InstActivation" and memref.startswith("x8"):
            ch = int(memref[2 : memref.index("_")])
            signs[ch] = ins
        elif tn == "InstTensorCopy" and memref.startswith("osb"):
            g = int(memref[3 : memref.index("_")])
            copies.setdefault(g, []).append(ins)

    assert sorted(xf_subs) == list(range(N_CH)) and all(
        len(v) == N_SUB for v in xf_subs.values()
    ), {k: len(v) for k, v in xf_subs.items()}
    assert sorted(signs) == list(range(N_CH))
    assert sorted(copies) == list(range(N_GRP)) and all(
        len(v) == 8 for v in copies.values()
    )
    assert sorted(stores) == list(range(N_GRP)) and all(
        len(v) == 2 for v in stores.values()
    )
    assert len(w_loads) == 1

    def wait_on(producer_ins):
        sem_name, sem_id, v = upd_after[producer_ins.name]
        return mybir.SyncWait(
            sync_type="semaphore",
            id=sem_id,
            ant_name=sem_name,
            wait_mode="sem-ge-imm",
            wait_value=v,
        )

    def keep_engine_waits(ins):
        return [
            w
            for w in (ins.sync_info.on_wait or [])
            if not (
                w.ant_name.startswith("DMAHW")
                or w.ant_name.startswith("DMASW")
                or w.ant_name.startswith("Activation")
                or w.ant_name.startswith("DVE")
            )
        ]

    def set_waits(ins, producers, extra=()):
        si = ins.sync_info
        waits = [wait_on(p) for p in producers if p is not None] + list(extra)
        lo = lane_order.get(ins.name)
        if lo is not None:
            waits.append(lo)
        ins.sync_info = mybir.SyncInfo(
            on_wait=waits, on_update=list(si.on_update or [])
        )

    for ch in range(N_CH):
        for k, ins in enumerate(xf_subs[ch]):
            deps = []
            if ch > 0:
                deps.append(xf_subs[ch - 1][k])
            if ch >= n_x_bufs:
                deps.append(signs[ch - n_x_bufs])
            set_waits(ins, deps)
    set_waits(w_loads[0], [])
    for ch in range(N_CH):
        set_waits(signs[ch], xf_subs[ch], extra=keep_engine_waits(signs[ch]))
    for g in range(N_GRP):
        for ins in copies[g]:
            set_waits(ins, [], extra=keep_engine_waits(ins))
        for ins in stores[g]:
            set_waits(ins, [copies[g][-1]])


def _legalize_dma_waits(nc):
    """Walrus caps in-struct sem waits (DMA_DIRECT2D takes 1, DMACopy 2).

    Tile's sem assignment is not transitively minimal and can emit 2-4 waits
    on DMA instructions. Hoist the excess into InstEventSemaphore wait-only
    instructions inserted just before the DMA on its triggering queue. This
    is sound: the queue executes the hoisted wait strictly before pushing the
    DMA descriptor, so the dependency is enforced (more conservatively) at
    trigger time instead of ring-pop time.
    """
    import concourse.mybir as mybir

    limits = {
        "InstDmaTransposeAnt": 1,
        "InstDMACopy": 1,
        "InstTensorCopy": 1,
        "InstActivation": 1,
        "InstMatmult": 1,
        "InstLdweights": 1,
        "InstMemset": 1,
        "InstTensorTensor": 1,
        "InstDrain": 1,
    }
    n_hoisted = 0
    for f in nc.m.functions:
        for bb in f.blocks:
            new_list = []
            for ins in bb.instructions:
                lim = limits.get(type(ins).__name__)
                si = getattr(ins, "sync_info", None)
                waits = list(si.on_wait) if si is not None and si.on_wait else []
                if lim is not None and len(waits) > lim:
                    # keep data-producer (engine-sem) waits in-struct first,
                    # then the freshest DMA-lane waits; hoist the rest
                    def keep_rank(w):
                        is_lane = w.ant_name.startswith(
                            "DMAHW"
                        ) or w.ant_name.startswith("DMASW")
                        return (1 if is_lane else 0, -w.wait_value)

                    waits_sorted = sorted(waits, key=keep_rank)
                    keep, hoist = waits_sorted[:lim], waits_sorted[lim:]
                    for ci in range(0, len(hoist), 2):
                        chunk = hoist[ci : ci + 2]
                        ev = mybir.InstEventSemaphore(
                            name=f"{ins.name}-prewait{ci // 2}",
                            engine=ins.engine,
                            ins=[],
                            outs=[],
                            sync_info=mybir.SyncInfo(on_wait=chunk, on_update=[]),
                        )
                        nc.inst_map[ev.name] = ev
                        new_list.append(ev)
                        n_hoisted += len(chunk)
                    ins.sync_info = mybir.SyncInfo(
                        on_wait=keep, on_update=list(si.on_update or [])
                    )
                new_list.append(ins)
            bb.instructions[:] = new_list
    return n_hoisted


def _build_nc():
    import concourse.bass as bass
    import concourse.mybir as mybir
    from concourse import tile

    nc = bass.Bass("TRN2", target_bir_lowering=False, num_swdge_queues=4)
    x_d = nc.dram_tensor(
        "x", [N_CH * P, 8 * MC], mybir.dt.float32, kind="ExternalInput"
    )
    w_d = nc.dram_tensor("W", [P, 8 * N], mybir.dt.float8e4, kind="ExternalInput")
    out_d = nc.dram_tensor(
        "out", [M_PER_CORE, N], mybir.dt.float16, kind="ExternalOutput"
    )
    with tile.TileContext(nc) as tc:
        build_binary_linear(tc, out_d.ap(), x_d.ap(), w_d.ap())
    _rewire_waits(nc, n_x_bufs=4)
    _legalize_dma_waits(nc)
    return nc


_cached = {}


def _get_nc():
    if "nc" not in _cached:
        _cached["nc"] = _build_nc()
    return _cached["nc"]


def kernel(x, W, _trace=False):
    from concourse import bass_utils

    import ml_dtypes

    xf = np.asarray(x, dtype=np.float32).reshape(M_TOTAL, K)
    # host re-layout (pure permutation): per core [ (g, b0, p), (j, c, t', u) ]
    # with m = 2048*core + 512g + 4u + 2b0 + t' and i = 256j + 128c + p
    T = xf.reshape(N_CORES, 4, P, 2, 2, 4, 2, P)  # (core, g, u, b0, t', j, c, p)
    xh = np.ascontiguousarray(T.transpose(0, 1, 3, 7, 5, 6, 4, 2)).reshape(
        N_CORES, N_CH * P, 8 * MC
    )
    # pack sign(W) fp8: wq[p, (j, c, o)] = sign(W)[o, 256j + 128c + p]
    sT = np.sign(np.asarray(W, dtype=np.float32)).T.astype(ml_dtypes.float8_e4m3)
    wq = np.ascontiguousarray(
        sT.reshape(4, 2, P, N).transpose(2, 0, 1, 3)
    ).reshape(P, 8 * N)
    in_maps = [{"x": xh[i], "W": wq} for i in range(N_CORES)]
    nc = _get_nc()
    res = bass_utils.run_bass_kernel_spmd(
        nc, in_maps, core_ids=list(range(N_CORES)), trace=_trace
    )
    out = np.concatenate([r["out"] for r in res.results], axis=0)
    out = out.astype(np.float32).reshape(4, 4096, N)
    if _trace:
        kernel.last_results = res
    return out


# revision 11
# speedup vs baseline: 1.2074x; 1.1209x over previous
"""BinaryLinear Trainium2 kernel: out = sign(x) @ sign(W).T

x: (4, 4096, 1024) f32, W: (1024, 1024) f32 -> out (4, 4096, 1024) f32.

Strategy (8 NeuronCores, data-parallel over flattened batch*seq):
  - Each core gets a [2048, 1024] row-shard of x and the full W.
  - x is re-laid-out on the host (pure permutation, no arithmetic) so the
    contraction index i lands on SBUF partitions directly: per core the DRAM
    tensor is [8 chunks * 128 p, (4 j, 2 c, 2 t', 128 u)] f32 with
    i = 256 j + 128 c + p and row m = 512 g + 4 u + 2 b0 + t' for chunk
    ch = 2 g + b0. This removes the on-chip transpose entirely and loads
    with 8 KiB-per-partition contiguous descriptors.
  - DMA descriptor generation is serialized per queue (~110-140 GB/s per
    queue), so each 1 MiB chunk load is split into 4 sub-DMAs pinned to the
    4 SWDGE queues (qPoolDynamic..qPoolDynamic3). Each queue is FIFO, so
    chunk completions stay ordered without explicit chaining while all 4
    descgen streams run in parallel (~450 GB/s issue capability, HBM-bound).
  - Per chunk (256 rows): ACT Sign (f32 -> fp8e4, +-1/0 exact) -> 16 fp8
    DoubleRow matmuls (K=256 each) accumulating [128 m, 512 o] PSUM tiles
    -> DVE copy PSUM -> SBUF as float16.
  - Outputs are exact integers |v| <= 1024, representable exactly in fp16,
    so stores are half-width; the host upcasts to f32. The 4-way row
    interleave (m = 512g + 4u + b) makes each partition hold 4 adjacent
    DRAM rows, i.e. 8 KiB store descriptors. Stores cover one 512-row
    group (two chunks) and are split into 2 sub-DMAs on the SP and
    Activation HWDGE queues so they never block a load trigger, with the
    Act-queue store triggers reordered after all signs so a store's
    copy-wait can never stall a sign.
  - W is repacked once on the host: wq[p, (j, c, o)] = sign(W)[o, i] fp8;
    loaded as 2 half-DMAs on the SP/Act HWDGE queues concurrently with the
    first x chunk. A dummy 1-element Sign activation with no dependencies
    preloads the ACT function table during the preamble.

All arithmetic is exact: sign values are +-1/0 (exact in fp8e4), the PE
accumulates in fp32, and |out| <= 1024 is exact in fp16.
"""

import numpy as np

P = 128
K = 1024  # in_features
N = 1024  # out_features
N_CORES = 8
M_TOTAL = 4 * 4096
M_PER_CORE = M_TOTAL // N_CORES
MC = 256  # rows per chunk
N_CH = M_PER_CORE // MC
N_SUB = 4  # load sub-DMAs per chunk (= SWDGE queue count)
N_GRP = N_CH // 2  # 512-row store groups
X_BUFS = 3


def build_binary_linear(tc, out, x, w):
    """Emit the per-core Tile kernel.

    out: DRAM [M_PER_CORE, N] f16, x: DRAM [N_CH*P, 8*MC] f32 (host-packed),
    w: DRAM [P, 8*N] fp8 (host-packed).
    """
    import concourse.mybir as mybir

    nc = tc.nc
    f32 = mybir.dt.float32
    f16 = mybir.dt.float16
    fp8 = mybir.dt.float8e4
    Sign = mybir.ActivationFunctionType.Sign
    DR = mybir.MatmulPerfMode.DoubleRow
    PQ = P // N_SUB

    with (
        tc.tile_pool(name="wsb", bufs=1) as wpool,
        tc.tile_pool(name="xin", bufs=X_BUFS) as xin_pool,
        tc.tile_pool(name="x8p", bufs=3) as x8_pool,
        tc.tile_pool(name="osb", bufs=4) as out_pool,
        tc.tile_pool(name="ps", bufs=2, space="PSUM") as psum_pool,
    ):
        # Preload the ACT Sign table during the preamble: a 1-partition,
        # 8-element Sign with no data dependencies.
        dumf = wpool.tile([1, 8], f32, name="dumf")
        dum8 = wpool.tile([1, 8], fp8, name="dum8")
        nc.vector.memset(dumf, 0.0)
        nc.scalar.activation(out=dum8, in_=dumf, func=Sign)

        # ---- W: host-packed fp8 [128, 8*1024]; wq[p, (j, c, o)]
        # = sign(W)[o, i] with i = 256j + 128c + p. Two half-DMAs on the
        # SP / Act HWDGE queues. ----
        wT = wpool.tile([P, 8 * N], fp8, name="wT")
        nc.sync.dma_start(out=wT[: P // 2, :], in_=w[: P // 2, :])
        nc.scalar.dma_start(out=wT[P // 2 :, :], in_=w[P // 2 :, :])
        w4 = wT.rearrange("p (j c o) -> p j c o", j=4, c=2)

        osbs = {}
        for ch in range(N_CH):
            g, b0 = divmod(ch, 2)
            xf = xin_pool.tile([P, 8 * MC], f32, tag="xf", name=f"xf{ch}")
            for q in range(N_SUB):
                inst = nc.gpsimd.dma_start(
                    out=xf[PQ * q : PQ * (q + 1), :],
                    in_=x[ch * P + PQ * q : ch * P + PQ * (q + 1), :],
                )
                inst.ins.queue = f"qPoolDynamic{q or ''}"
            x8 = x8_pool.tile([P, 8 * MC], fp8, tag="x8", name=f"x8{ch}")
            nc.scalar.activation(out=x8, in_=xf, func=Sign)
            x84 = x8.rearrange("p (j c m) -> p j c m", j=4, c=2)

            if b0 == 0:
                osbs[g] = out_pool.tile([P, 4 * N], f16, tag="osb", name=f"osb{g}")
            osb2 = osbs[g].rearrange("p (b o) -> p b o", b=4)
            for t in range(MC // P):
                ps = [
                    psum_pool.tile([P, 512], f32, tag=f"ps{t}{h}", name=f"ps{t}{h}")
                    for h in range(2)
                ]
                for j in range(4):
                    lhsT = x84[:, j, :, t * P : (t + 1) * P]
                    for h in range(2):
                        nc.tensor.matmul(
                            ps[h],
                            lhsT=lhsT,
                            rhs=w4[:, j, :, h * 512 : (h + 1) * 512],
                            start=(j == 0),
                            stop=(j == 3),
                            perf_mode=DR,
                        )
                for h in range(2):
                    nc.vector.tensor_copy(
                        out=osb2[:, 2 * b0 + t, h * 512 : (h + 1) * 512], in_=ps[h]
                    )
            if b0 == 1:
                # out rows 512g + 4p + b  <-  osb[p, b, o]; two sub-DMAs,
                # one on SP, one on Act (each HWDGE queue is descgen-bound).
                for q, eng in enumerate((nc.sync, nc.scalar)):
                    r0 = 512 * g + 256 * q
                    eng.dma_start(
                        out=out[r0 : r0 + 256].rearrange("(p b) o -> p (b o)", b=4),
                        in_=osbs[g][64 * q : 64 * (q + 1), :],
                    )


def _rewire_waits(nc):
    """Reorder Act-queue store triggers after all signs, then replace Tile's
    conservative / lane-aliased DMA waits with exact producer-based waits.

      xf[ch][k]  <- sign[ch - X_BUFS] (xf-slot WAR); queue FIFO orders the
                    per-queue descgen streams, no chaining needed
      w halves   <- (nothing; first on their HWDGE queues)
      sign[ch]   <- all 4 xf[ch] sub completions (RAW) + keep Tile's PE
                    wait (x8-slot WAR)
      copy[...]  <- keep Tile's PE wait only (psum RAW; osb pool has one
                    buffer per group, no WAR)
      store[g,q] <- last copy of group g (RAW)

    Waits are emitted as (producer's update-sem >= cumulative value after
    it); lane-order waits keep same-sem DMA updates ordered so >= waits
    cannot be satisfied by a later DMA that shares the semaphore.
    """
    import concourse.mybir as mybir

    # -- pass 0: move Act-engine store DMAs after the last InstActivation --
    for f in nc.m.functions:
        for bb in f.blocks:
            ins_list = bb.instructions
            act_stores = [
                i
                for i in ins_list
                if type(i).__name__ == "InstDMACopy"
                and str(i.engine).endswith("Activation")
                and str(i.outs[0].memref).startswith("out")
            ]
            if not act_stores:
                continue
            rest = [i for i in ins_list if i not in act_stores]
            last_act = max(
                idx
                for idx, i in enumerate(rest)
                if type(i).__name__ == "InstActivation"
            )
            bb.instructions[:] = (
                rest[: last_act + 1] + act_stores + rest[last_act + 1 :]
            )

    insts = []
    for f in nc.m.functions:
        for bb in f.blocks:
            insts.extend(bb.instructions)

    cum = {}
    upd_after = {}  # inst name -> (sem_name, sem_id, cum_value_after)
    lane_order = {}  # inst name -> SyncWait enforcing same-lane completion order
    xf_subs = {}  # ch -> [inst]
    signs = {}  # ch -> inst
    copies = {}  # g -> [inst]
    stores = {}  # g -> [inst]
    w_loads = []
    for ins in insts:
        si = getattr(ins, "sync_info", None)
        if si is None:
            continue
        for u in si.on_update or []:
            prev = cum.get(u.ant_name, 0)
            if prev > 0 and (
                u.ant_name.startswith("DMAHW") or u.ant_name.startswith("DMASW")
            ):
                lane_order[ins.name] = mybir.SyncWait(
                    sync_type="semaphore",
                    id=u.id,
                    ant_name=u.ant_name,
                    wait_mode="sem-ge-imm",
                    wait_value=prev,
                )
            cum[u.ant_name] = prev + u.update_value
            upd_after[ins.name] = (u.ant_name, u.id, cum[u.ant_name])
        memref = str(getattr(ins.outs[0], "memref", "")) if ins.outs else ""
        tn = type(ins).__name__
        if tn == "InstDMACopy" and memref.startswith("xf"):
            ch = int(memref[2 : memref.index("_")])
            xf_subs.setdefault(ch, []).append(ins)
        elif tn == "InstDMACopy" and memref.startswith("wT"):
            w_loads.append(ins)
        elif tn == "InstDMACopy" and memref.startswith("out"):
            off = int(ins.outs[0].offset)  # in f16 elements
            g = off // (512 * N)
            stores.setdefault(g, []).append(ins)
        elif tn == "InstActivation" and memref.startswith("x8"):
            ch = int(memref[2 : memref.index("_")])
            signs[ch] = ins
        elif tn == "InstTensorCopy" and memref.startswith("osb"):
            g = int(memref[3 : memref.index("_")])
            copies.setdefault(g, []).append(ins)

    assert sorted(xf_subs) == list(range(N_CH)) and all(
        len(v) == N_SUB for v in xf_subs.values()
    ), {k: len(v) for k, v in xf_subs.items()}
    assert sorted(signs) == list(range(N_CH))
    assert sorted(copies) == list(range(N_GRP)) and all(
        len(v) == 8 for v in copies.values()
    )
    assert sorted(stores) == list(range(N_GRP)) and all(
        len(v) == 2 for v in stores.values()
    )
    assert len(w_loads) == 2

    def wait_on(producer_ins):
        sem_name, sem_id, v = upd_after[producer_ins.name]
        return mybir.SyncWait(
            sync_type="semaphore",
            id=sem_id,
            ant_name=sem_name,
            wait_mode="sem-ge-imm",
            wait_value=v,
        )

    def keep_engine_waits(ins):
        return [
            w
            for w in (ins.sync_info.on_wait or [])
            if not (
                w.ant_name.startswith("DMAHW")
                or w.ant_name.startswith("DMASW")
                or w.ant_name.startswith("Activation")
                or w.ant_name.startswith("DVE")
            )
        ]

    def set_waits(ins, producers, extra=()):
        si = ins.sync_info
        waits = [wait_on(p) for p in producers if p is not None] + list(extra)
        lo = lane_order.get(ins.name)
        if lo is not None:
            waits.append(lo)
        ins.sync_info = mybir.SyncInfo(
            on_wait=waits, on_update=list(si.on_update or [])
        )

    for ch in range(N_CH):
        for ins in xf_subs[ch]:
            set_waits(ins, [signs[ch - X_BUFS]] if ch >= X_BUFS else [])
    for ins in w_loads:
        set_waits(ins, [])
    for ch in range(N_CH):
        set_waits(signs[ch], xf_subs[ch], extra=keep_engine_waits(signs[ch]))
    for g in range(N_GRP):
        for ins in copies[g]:
            set_waits(ins, [], extra=keep_engine_waits(ins))
        for ins in stores[g]:
            set_waits(ins, [copies[g][-1]])


def _legalize_dma_waits(nc):
    """Walrus caps in-struct sem waits (DMA_DIRECT2D takes 1, DMACopy 2).

    Tile's sem assignment is not transitively minimal and can emit 2-4 waits
    on DMA instructions. Hoist the excess into InstEventSemaphore wait-only
    instructions inserted just before the DMA on its triggering queue. This
    is sound: the queue executes the hoisted wait strictly before pushing the
    DMA descriptor, so the dependency is enforced (more conservatively) at
    trigger time instead of ring-pop time.
    """
    import concourse.mybir as mybir

    limits = {
        "InstDmaTransposeAnt": 1,
        "InstDMACopy": 1,
        "InstTensorCopy": 1,
        "InstActivation": 1,
        "InstMatmult": 1,
        "InstLdweights": 1,
        "InstMemset": 1,
        "InstTensorTensor": 1,
        "InstDrain": 1,
    }
    n_hoisted = 0
    for f in nc.m.functions:
        for bb in f.blocks:
            new_list = []
            for ins in bb.instructions:
                lim = limits.get(type(ins).__name__)
                si = getattr(ins, "sync_info", None)
                waits = list(si.on_wait) if si is not None and si.on_wait else []
                if lim is not None and len(waits) > lim:
                    # keep data-producer (engine-sem) waits in-struct first,
                    # then the freshest DMA-lane waits; hoist the rest
                    def keep_rank(w):
                        is_lane = w.ant_name.startswith(
                            "DMAHW"
                        ) or w.ant_name.startswith("DMASW")
                        return (1 if is_lane else 0, -w.wait_value)

                    waits_sorted = sorted(waits, key=keep_rank)
                    keep, hoist = waits_sorted[:lim], waits_sorted[lim:]
                    for ci in range(0, len(hoist), 2):
                        chunk = hoist[ci : ci + 2]
                        ev = mybir.InstEventSemaphore(
                            name=f"{ins.name}-prewait{ci // 2}",
                            engine=ins.engine,
                            ins=[],
                            outs=[],
                            sync_info=mybir.SyncInfo(on_wait=chunk, on_update=[]),
                        )
                        nc.inst_map[ev.name] = ev
                        new_list.append(ev)
                        n_hoisted += len(chunk)
                    ins.sync_info = mybir.SyncInfo(
                        on_wait=keep, on_update=list(si.on_update or [])
                    )
                new_list.append(ins)
            bb.instructions[:] = new_list
    return n_hoisted


def _build_nc():
    import concourse.bass as bass
    import concourse.mybir as mybir
    from concourse import tile

    nc = bass.Bass("TRN2", target_bir_lowering=False, num_swdge_queues=4)
    x_d = nc.dram_tensor(
        "x", [N_CH * P, 8 * MC], mybir.dt.float32, kind="ExternalInput"
    )
    w_d = nc.dram_tensor("W", [P, 8 * N], mybir.dt.float8e4, kind="ExternalInput")
    out_d = nc.dram_tensor(
        "out", [M_PER_CORE, N], mybir.dt.float16, kind="ExternalOutput"
    )
    with tile.TileContext(nc) as tc:
        build_binary_linear(tc, out_d.ap(), x_d.ap(), w_d.ap())
    _rewire_waits(nc)
    _legalize_dma_waits(nc)
    return nc


_cached = {}


def _get_nc():
    if "nc" not in _cached:
        _cached["nc"] = _build_nc()
    return _cached["nc"]


def kernel(x, W, _trace=False):
    from concourse import bass_utils

    import ml_dtypes

    xf = np.asarray(x, dtype=np.float32).reshape(M_TOTAL, K)
    # host re-layout (pure permutation): per core [ (g, b0, p), (j, c, t', u) ]
    # with m = 2048*core + 512g + 4u + 2b0 + t' and i = 256j + 128c + p
    T = xf.reshape(N_CORES, 4, P, 2, 2, 4, 2, P)  # (core, g, u, b0, t', j, c, p)
    xh = np.ascontiguousarray(T.transpose(0, 1, 3, 7, 5, 6, 4, 2)).reshape(
        N_CORES, N_CH * P, 8 * MC
    )
    # pack sign(W) fp8: wq[p, (j, c, o)] = sign(W)[o, 256j + 128c + p]
    sT = np.sign(np.asarray(W, dtype=np.float32)).T.astype(ml_dtypes.float8_e4m3)
    wq = np.ascontiguousarray(
        sT.reshape(4, 2, P, N).transpose(2, 0, 1, 3)
    ).reshape(P, 8 * N)
    in_maps = [{"x": xh[i], "W": wq} for i in range(N_CORES)]
    nc = _get_nc()
    res = bass_utils.run_bass_kernel_spmd(
        nc, in_maps, core_ids=list(range(N_CORES)), trace=_trace
    )
    out = np.concatenate([r["out"] for r in res.results], axis=0)
    out = out.astype(np.float32).reshape(4, 4096, N)
    if _trace:
        kernel.last_results = res
    return out


# revision 12
# speedup vs baseline: 1.2428x; 1.0293x over previous
"""BinaryLinear Trainium2 kernel: out = sign(x) @ sign(W).T

x: (4, 4096, 1024) f32, W: (1024, 1024) f32 -> out (4, 4096, 1024) f32.

Strategy (8 NeuronCores, data-parallel over flattened batch*seq):
  - Each core gets a [2048, 1024] row-shard of x and the full W.
  - x is re-laid-out on the host (pure permutation, no arithmetic) so the
    contraction index i lands on SBUF partitions directly: per core the DRAM
    tensor is [8 chunks * 128 p, (4 j, 2 c, 2 t', 128 u)] f32 with
    i = 256 j + 128 c + p and row m = 512 g + 4 u + 2 b0 + t' for chunk
    ch = 2 g + b0. This removes the on-chip transpose entirely and loads
    with 8 KiB-per-partition contiguous descriptors.
  - DMA descriptor generation is serialized per queue (~110-140 GB/s per
    queue), so each 1 MiB chunk load is split into 4 sub-DMAs pinned to the
    4 SWDGE queues (qPoolDynamic..qPoolDynamic3). Each queue is FIFO, so
    chunk completions stay ordered without explicit chaining while all 4
    descgen streams run in parallel (~450 GB/s issue capability, HBM-bound).
  - Per chunk (256 rows): ACT Sign (f32 -> fp8e4, +-1/0 exact) -> 16 fp8
    DoubleRow matmuls (K=256 each) accumulating [128 m, 512 o] PSUM tiles
    -> DVE copy PSUM -> SBUF as float16.
  - Outputs are exact integers |v| <= 1024, representable exactly in fp16,
    so stores are half-width; the host upcasts to f32. The 4-way row
    interleave (m = 512g + 4u + b) makes each partition hold 4 adjacent
    DRAM rows, i.e. 8 KiB store descriptors. Stores cover one 512-row
    group (two chunks) and are split into 2 sub-DMAs on the SP and
    Activation HWDGE queues so they never block a load trigger, with the
    Act-queue store triggers reordered after all signs so a store's
    copy-wait can never stall a sign.
  - W is repacked once on the host: wq[p, (j, c, o)] = sign(W)[o, i] fp8;
    loaded as 2 half-DMAs on the SP/Act HWDGE queues concurrently with the
    first x chunk. A dummy 1-element Sign activation with no dependencies
    preloads the ACT function table during the preamble.

All arithmetic is exact: sign values are +-1/0 (exact in fp8e4), the PE
accumulates in fp32, and |out| <= 1024 is exact in fp16.
"""

import numpy as np

P = 128
K = 1024  # in_features
N = 1024  # out_features
N_CORES = 8
M_TOTAL = 4 * 4096
M_PER_CORE = M_TOTAL // N_CORES
MC = 128  # rows per chunk
N_CH = M_PER_CORE // MC
N_GRP = N_CH // 4  # 512-row store groups (4 chunks each)
X_BUFS = 6


def build_binary_linear(tc, out, x, w):
    """Emit the per-core Tile kernel.

    out: DRAM [M_PER_CORE, N] f16, x: DRAM [N_CH*P, 8*MC] f32 (host-packed),
    w: DRAM [P, 8*N] fp8 (host-packed).
    """
    import concourse.mybir as mybir

    nc = tc.nc
    f32 = mybir.dt.float32
    f16 = mybir.dt.float16
    fp8 = mybir.dt.float8e4
    Sign = mybir.ActivationFunctionType.Sign
    DR = mybir.MatmulPerfMode.DoubleRow

    with (
        tc.tile_pool(name="wsb", bufs=1) as wpool,
        tc.tile_pool(name="xin", bufs=X_BUFS) as xin_pool,
        tc.tile_pool(name="x8p", bufs=3) as x8_pool,
        tc.tile_pool(name="osb", bufs=4) as out_pool,
        tc.tile_pool(name="ps", bufs=4, space="PSUM") as psum_pool,
    ):
        # Preload the ACT Sign table during the preamble: a 1-partition,
        # 8-element Sign with no data dependencies.
        dumf = wpool.tile([1, 8], f32, name="dumf")
        dum8 = wpool.tile([1, 8], fp8, name="dum8")
        nc.vector.memset(dumf, 0.0)
        nc.scalar.activation(out=dum8, in_=dumf, func=Sign)

        # ---- W: host-packed fp8 [128, 8*1024]; wq[p, (j, c, o)]
        # = sign(W)[o, i] with i = 256j + 128c + p. Two half-DMAs on the
        # SP / Act HWDGE queues. ----
        wT = wpool.tile([P, 8 * N], fp8, name="wT")
        nc.sync.dma_start(out=wT[: P // 2, :], in_=w[: P // 2, :])
        nc.scalar.dma_start(out=wT[P // 2 :, :], in_=w[P // 2 :, :])
        w4 = wT.rearrange("p (j c o) -> p j c o", j=4, c=2)

        osbs = {}
        for ch in range(N_CH):
            g, b0 = divmod(ch, 4)
            xf = xin_pool.tile([P, 8 * MC], f32, tag="xf", name=f"xf{ch}")
            inst = nc.gpsimd.dma_start(out=xf, in_=x[ch * P : (ch + 1) * P, :])
            inst.ins.queue = f"qPoolDynamic{(ch % 4) or ''}"
            x8 = x8_pool.tile([P, 8 * MC], fp8, tag="x8", name=f"x8{ch}")
            nc.scalar.activation(out=x8, in_=xf, func=Sign)
            x84 = x8.rearrange("p (j c m) -> p j c m", j=4, c=2)

            if b0 == 0:
                osbs[g] = out_pool.tile([P, 4 * N], f16, tag="osb", name=f"osb{g}")
            osb2 = osbs[g].rearrange("p (b o) -> p b o", b=4)
            ps = [
                psum_pool.tile([P, 512], f32, tag=f"ps{h}", name=f"ps{h}")
                for h in range(2)
            ]
            for j in range(4):
                lhsT = x84[:, j, :, :]
                for h in range(2):
                    nc.tensor.matmul(
                        ps[h],
                        lhsT=lhsT,
                        rhs=w4[:, j, :, h * 512 : (h + 1) * 512],
                        start=(j == 0),
                        stop=(j == 3),
                        perf_mode=DR,
                    )
            for h in range(2):
                nc.vector.tensor_copy(
                    out=osb2[:, b0, h * 512 : (h + 1) * 512], in_=ps[h]
                )
            if b0 == 3:
                # out rows 512g + 4p + b  <-  osb[p, b, o]; two sub-DMAs,
                # one on SP, one on Act (each HWDGE queue is descgen-bound).
                for q, eng in enumerate((nc.sync, nc.scalar)):
                    r0 = 512 * g + 256 * q
                    eng.dma_start(
                        out=out[r0 : r0 + 256].rearrange("(p b) o -> p (b o)", b=4),
                        in_=osbs[g][64 * q : 64 * (q + 1), :],
                    )


def _rewire_waits(nc):
    """Reorder Act-queue store triggers after all signs, then replace Tile's
    conservative / lane-aliased DMA waits with exact producer-based waits.

      xf[ch]     <- sign[ch - X_BUFS] (xf-slot WAR)
      w halves   <- (nothing; first on their HWDGE queues)
      sign[ch]   <- all 4 xf[ch] sub completions (RAW) + keep Tile's PE
                    wait (x8-slot WAR)
      copy[...]  <- keep Tile's PE wait only (psum RAW; osb pool has one
                    buffer per group, no WAR)
      store[g,q] <- last copy of group g (RAW)

    Waits are emitted as (producer's update-sem >= cumulative value after
    it); lane-order waits keep same-sem DMA updates ordered so >= waits
    cannot be satisfied by a later DMA that shares the semaphore.
    """
    import concourse.mybir as mybir

    # -- pass 0: move Act-engine store DMAs after the last InstActivation --
    for f in nc.m.functions:
        for bb in f.blocks:
            ins_list = bb.instructions
            act_stores = [
                i
                for i in ins_list
                if type(i).__name__ == "InstDMACopy"
                and str(i.engine).endswith("Activation")
                and str(i.outs[0].memref).startswith("out")
            ]
            if not act_stores:
                continue
            rest = [i for i in ins_list if i not in act_stores]
            last_act = max(
                idx
                for idx, i in enumerate(rest)
                if type(i).__name__ == "InstActivation"
            )
            bb.instructions[:] = (
                rest[: last_act + 1] + act_stores + rest[last_act + 1 :]
            )

    insts = []
    for f in nc.m.functions:
        for bb in f.blocks:
            insts.extend(bb.instructions)

    cum = {}
    upd_after = {}  # inst name -> (sem_name, sem_id, cum_value_after)
    lane_order = {}  # inst name -> SyncWait enforcing same-lane completion order
    xf_subs = {}  # ch -> [inst]
    signs = {}  # ch -> inst
    copies = {}  # g -> [inst]
    stores = {}  # g -> [inst]
    w_loads = []
    for ins in insts:
        si = getattr(ins, "sync_info", None)
        if si is None:
            continue
        for u in si.on_update or []:
            prev = cum.get(u.ant_name, 0)
            if prev > 0 and (
                u.ant_name.startswith("DMAHW") or u.ant_name.startswith("DMASW")
            ):
                lane_order[ins.name] = mybir.SyncWait(
                    sync_type="semaphore",
                    id=u.id,
                    ant_name=u.ant_name,
                    wait_mode="sem-ge-imm",
                    wait_value=prev,
                )
            cum[u.ant_name] = prev + u.update_value
            upd_after[ins.name] = (u.ant_name, u.id, cum[u.ant_name])
        memref = str(getattr(ins.outs[0], "memref", "")) if ins.outs else ""
        tn = type(ins).__name__
        if tn == "InstDMACopy" and memref.startswith("xf"):
            ch = int(memref[2 : memref.index("_")])
            xf_subs.setdefault(ch, []).append(ins)
        elif tn == "InstDMACopy" and memref.startswith("wT"):
            w_loads.append(ins)
        elif tn == "InstDMACopy" and memref.startswith("out"):
            off = int(ins.outs[0].offset)  # in f16 elements
            g = off // (512 * N)
            stores.setdefault(g, []).append(ins)
        elif tn == "InstActivation" and memref.startswith("x8"):
            ch = int(memref[2 : memref.index("_")])
            signs[ch] = ins
        elif tn == "InstTensorCopy" and memref.startswith("osb"):
            g = int(memref[3 : memref.index("_")])
            copies.setdefault(g, []).append(ins)

    assert sorted(xf_subs) == list(range(N_CH)) and all(
        len(v) == 1 for v in xf_subs.values()
    ), {k: len(v) for k, v in xf_subs.items()}
    assert sorted(signs) == list(range(N_CH))
    assert sorted(copies) == list(range(N_GRP)) and all(
        len(v) == 8 for v in copies.values()
    )
    assert sorted(stores) == list(range(N_GRP)) and all(
        len(v) == 2 for v in stores.values()
    )
    assert len(w_loads) == 2

    def wait_on(producer_ins):
        sem_name, sem_id, v = upd_after[producer_ins.name]
        return mybir.SyncWait(
            sync_type="semaphore",
            id=sem_id,
            ant_name=sem_name,
            wait_mode="sem-ge-imm",
            wait_value=v,
        )

    def keep_engine_waits(ins):
        return [
            w
            for w in (ins.sync_info.on_wait or [])
            if not (
                w.ant_name.startswith("DMAHW")
                or w.ant_name.startswith("DMASW")
                or w.ant_name.startswith("Activation")
                or w.ant_name.startswith("DVE")
            )
        ]

    def set_waits(ins, producers, extra=()):
        si = ins.sync_info
        waits = [wait_on(p) for p in producers if p is not None] + list(extra)
        lo = lane_order.get(ins.name)
        if lo is not None:
            waits.append(lo)
        ins.sync_info = mybir.SyncInfo(
            on_wait=waits, on_update=list(si.on_update or [])
        )

    for ch in range(N_CH):
        for ins in xf_subs[ch]:
            set_waits(ins, [signs[ch - X_BUFS]] if ch >= X_BUFS else [])
    for ins in w_loads:
        set_waits(ins, [])
    for ch in range(N_CH):
        set_waits(signs[ch], xf_subs[ch], extra=keep_engine_waits(signs[ch]))
    for g in range(N_GRP):
        for ins in copies[g]:
            set_waits(ins, [], extra=keep_engine_waits(ins))
        for ins in stores[g]:
            set_waits(ins, [copies[g][-1]])


def _legalize_dma_waits(nc):
    """Walrus caps in-struct sem waits (DMA_DIRECT2D takes 1, DMACopy 2).

    Tile's sem assignment is not transitively minimal and can emit 2-4 waits
    on DMA instructions. Hoist the excess into InstEventSemaphore wait-only
    instructions inserted just before the DMA on its triggering queue. This
    is sound: the queue executes the hoisted wait strictly before pushing the
    DMA descriptor, so the dependency is enforced (more conservatively) at
    trigger time instead of ring-pop time.
    """
    import concourse.mybir as mybir

    limits = {
        "InstDmaTransposeAnt": 1,
        "InstDMACopy": 1,
        "InstTensorCopy": 1,
        "InstActivation": 1,
        "InstMatmult": 1,
        "InstLdweights": 1,
        "InstMemset": 1,
        "InstTensorTensor": 1,
        "InstDrain": 1,
    }
    n_hoisted = 0
    for f in nc.m.functions:
        for bb in f.blocks:
            new_list = []
            for ins in bb.instructions:
                lim = limits.get(type(ins).__name__)
                si = getattr(ins, "sync_info", None)
                waits = list(si.on_wait) if si is not None and si.on_wait else []
                if lim is not None and len(waits) > lim:
                    # keep data-producer (engine-sem) waits in-struct first,
                    # then the freshest DMA-lane waits; hoist the rest
                    def keep_rank(w):
                        is_lane = w.ant_name.startswith(
                            "DMAHW"
                        ) or w.ant_name.startswith("DMASW")
                        return (1 if is_lane else 0, -w.wait_value)

                    waits_sorted = sorted(waits, key=keep_rank)
                    keep, hoist = waits_sorted[:lim], waits_sorted[lim:]
                    for ci in range(0, len(hoist), 2):
                        chunk = hoist[ci : ci + 2]
                        ev = mybir.InstEventSemaphore(
                            name=f"{ins.name}-prewait{ci // 2}",
                            engine=ins.engine,
                            ins=[],
                            outs=[],
                            sync_info=mybir.SyncInfo(on_wait=chunk, on_update=[]),
                        )
                        nc.inst_map[ev.name] = ev
                        new_list.append(ev)
                        n_hoisted += len(chunk)
                    ins.sync_info = mybir.SyncInfo(
                        on_wait=keep, on_update=list(si.on_update or [])
                    )
                new_list.append(ins)
            bb.instructions[:] = new_list
    return n_hoisted


def _build_nc():
    import concourse.bass as bass
    import concourse.mybir as mybir
    from concourse import tile

    nc = bass.Bass("TRN2", target_bir_lowering=False, num_swdge_queues=4)
    x_d = nc.dram_tensor(
        "x", [N_CH * P, 8 * MC], mybir.dt.float32, kind="ExternalInput"
    )
    w_d = nc.dram_tensor("W", [P, 8 * N], mybir.dt.float8e4, kind="ExternalInput")
    out_d = nc.dram_tensor(
        "out", [M_PER_CORE, N], mybir.dt.float16, kind="ExternalOutput"
    )
    with tile.TileContext(nc) as tc:
        build_binary_linear(tc, out_d.ap(), x_d.ap(), w_d.ap())
    _rewire_waits(nc)
    _legalize_dma_waits(nc)
    return nc


_cached = {}


def _get_nc():
    if "nc" not in _cached:
        _cached["nc"] = _build_nc()
    return _cached["nc"]


def kernel(x, W, _trace=False):
    from concourse import bass_utils

    import ml_dtypes

    xf = np.asarray(x, dtype=np.float32).reshape(M_TOTAL, K)
    # host re-layout (pure permutation): per core [ (g, b0, p), (j, c, u) ]
    # with m = 2048*core + 512g + 4u + b0 and i = 256j + 128c + p
    T = xf.reshape(N_CORES, 4, P, 4, 4, 2, P)  # (core, g, u, b0, j, c, p)
    xh = np.ascontiguousarray(T.transpose(0, 1, 3, 6, 4, 5, 2)).reshape(
        N_CORES, N_CH * P, 8 * MC
    )
    # pack sign(W) fp8: wq[p, (j, c, o)] = sign(W)[o, 256j + 128c + p]
    sT = np.sign(np.asarray(W, dtype=np.float32)).T.astype(ml_dtypes.float8_e4m3)
    wq = np.ascontiguousarray(
        sT.reshape(4, 2, P, N).transpose(2, 0, 1, 3)
    ).reshape(P, 8 * N)
    in_maps = [{"x": xh[i], "W": wq} for i in range(N_CORES)]
    nc = _get_nc()
    res = bass_utils.run_bass_kernel_spmd(
        nc, in_maps, core_ids=list(range(N_CORES)), trace=_trace
    )
    out = np.concatenate([r["out"] for r in res.results], axis=0)
    out = out.astype(np.float32).reshape(4, 4096, N)
    if _trace:
        kernel.last_results = res
    return out


# revision 13
# speedup vs baseline: 1.3318x; 1.0717x over previous
"""BinaryLinear Trainium2 kernel: out = sign(x) @ sign(W).T

x: (4, 4096, 1024) f32, W: (1024, 1024) f32 -> out (4, 4096, 1024) f32.

Strategy (8 NeuronCores, data-parallel over flattened batch*seq):
  - Each core gets a [2048, 1024] row-shard of x and the full W.
  - x is re-laid-out on the host (pure permutation, no arithmetic) so the
    contraction index i lands on SBUF partitions directly: per core the DRAM
    tensor is [8 chunks * 128 p, (4 j, 2 c, 2 t', 128 u)] f32 with
    i = 256 j + 128 c + p and row m = 512 g + 4 u + 2 b0 + t' for chunk
    ch = 2 g + b0. This removes the on-chip transpose entirely and loads
    with 8 KiB-per-partition contiguous descriptors.
  - DMA descriptor generation is serialized per queue (~110-140 GB/s per
    queue), so each 1 MiB chunk load is split into 4 sub-DMAs pinned to the
    4 SWDGE queues (qPoolDynamic..qPoolDynamic3). Each queue is FIFO, so
    chunk completions stay ordered without explicit chaining while all 4
    descgen streams run in parallel (~450 GB/s issue capability, HBM-bound).
  - Per chunk (256 rows): ACT Sign (f32 -> fp8e4, +-1/0 exact) -> 16 fp8
    DoubleRow matmuls (K=256 each) accumulating [128 m, 512 o] PSUM tiles
    -> DVE copy PSUM -> SBUF as float16.
  - Outputs are exact integers |v| <= 1024, representable exactly in fp16,
    so stores are half-width; the host upcasts to f32. The 4-way row
    interleave (m = 512g + 4u + b) makes each partition hold 4 adjacent
    DRAM rows, i.e. 8 KiB store descriptors. Stores cover one 512-row
    group (two chunks) and are split into 2 sub-DMAs on the SP and
    Activation HWDGE queues so they never block a load trigger, with the
    Act-queue store triggers reordered after all signs so a store's
    copy-wait can never stall a sign.
  - W is repacked once on the host: wq[p, (j, c, o)] = sign(W)[o, i] fp8;
    loaded as 2 half-DMAs on the SP/Act HWDGE queues concurrently with the
    first x chunk. A dummy 1-element Sign activation with no dependencies
    preloads the ACT function table during the preamble.

All arithmetic is exact: sign values are +-1/0 (exact in fp8e4), the PE
accumulates in fp32, and |out| <= 1024 is exact in fp16.
"""

import numpy as np

P = 128
K = 1024  # in_features
N = 1024  # out_features
N_CORES = 8
M_TOTAL = 4 * 4096
M_PER_CORE = M_TOTAL // N_CORES
MC = 128  # rows per chunk
N_CH = M_PER_CORE // MC
N_GRP = N_CH // 4  # 512-row store groups (4 chunks each)
X_BUFS = 6


def build_binary_linear(tc, out, x, w):
    """Emit the per-core Tile kernel.

    out: DRAM [M_PER_CORE, N] f16, x: DRAM [N_CH*P, 8*MC] f32 (host-packed),
    w: DRAM [P, 8*N] fp8 (host-packed).
    """
    import concourse.mybir as mybir

    nc = tc.nc
    f32 = mybir.dt.float32
    f16 = mybir.dt.float16
    fp8 = mybir.dt.float8e4
    Sign = mybir.ActivationFunctionType.Sign
    DR = mybir.MatmulPerfMode.DoubleRow

    with (
        tc.tile_pool(name="wsb", bufs=1) as wpool,
        tc.tile_pool(name="xin", bufs=X_BUFS) as xin_pool,
        tc.tile_pool(name="x8p", bufs=4) as x8_pool,
        tc.tile_pool(name="osb", bufs=4) as out_pool,
        tc.tile_pool(name="ps", bufs=4, space="PSUM") as psum_pool,
    ):
        # Preload the ACT Sign table during the preamble: a 1-partition,
        # 8-element Sign with no data dependencies.
        dumf = wpool.tile([1, 8], f32, name="dumf")
        dum8 = wpool.tile([1, 8], fp8, name="dum8")
        nc.vector.memset(dumf, 0.0)
        nc.scalar.activation(out=dum8, in_=dumf, func=Sign)

        # ---- W: host-packed fp8 [128, 8*1024]; wq[p, (j, c, o)]
        # = sign(W)[o, i] with i = 256j + 128c + p. Two half-DMAs on the
        # SP / Act HWDGE queues. ----
        wT = wpool.tile([P, 8 * N], fp8, name="wT")
        nc.sync.dma_start(out=wT[: P // 2, :], in_=w[: P // 2, :])
        nc.scalar.dma_start(out=wT[P // 2 :, :], in_=w[P // 2 :, :])
        w4 = wT.rearrange("p (j c o) -> p j c o", j=4, c=2)

        osbs = {}
        for ch in range(N_CH):
            g, b0 = divmod(ch, 4)
            xf = xin_pool.tile([P, 8 * MC], f32, tag="xf", name=f"xf{ch}")
            inst = nc.gpsimd.dma_start(out=xf, in_=x[ch * P : (ch + 1) * P, :])
            inst.ins.queue = f"qPoolDynamic{(ch % 4) or ''}"
            x8 = x8_pool.tile([P, 8 * MC], fp8, tag="x8", name=f"x8{ch}")
            nc.scalar.activation(out=x8, in_=xf, func=Sign)
            x84 = x8.rearrange("p (j c m) -> p j c m", j=4, c=2)

            if b0 == 0:
                osbs[g] = out_pool.tile([P, 4 * N], f16, tag="osb", name=f"osb{g}")
            osb2 = osbs[g].rearrange("p (b o) -> p b o", b=4)
            ps = [
                psum_pool.tile([P, 512], f32, tag=f"ps{h}", name=f"ps{h}")
                for h in range(2)
            ]
            for j in range(4):
                lhsT = x84[:, j, :, :]
                for h in range(2):
                    nc.tensor.matmul(
                        ps[h],
                        lhsT=lhsT,
                        rhs=w4[:, j, :, h * 512 : (h + 1) * 512],
                        start=(j == 0),
                        stop=(j == 3),
                        perf_mode=DR,
                    )
            for h in range(2):
                nc.vector.tensor_copy(
                    out=osb2[:, b0, h * 512 : (h + 1) * 512], in_=ps[h]
                )
            if b0 == 3:
                # out rows 512g + 4p + b  <-  osb[p, b, o]; two sub-DMAs.
                # Mid-pipeline groups go on the Pool SWDGE queues (trigger
                # is ~0.6us and descgen is offloaded, so the ~9.5us DMA
                # completion latency hides behind the PE-paced pipeline);
                # the rewire pass repositions them after a load trigger
                # whose wait is later than the store's, so they never
                # block a load. The LAST group's store latency is exposed,
                # so it uses the engine-synchronous SP/Act HWDGE queues
                # (~3-4us completion) right after its copies.
                for q in range(2):
                    r0 = 512 * g + 256 * q
                    o_ap = out[r0 : r0 + 256].rearrange("(p b) o -> p (b o)", b=4)
                    i_ap = osbs[g][64 * q : 64 * (q + 1), :]
                    if g < N_GRP - 1:
                        inst = nc.gpsimd.dma_start(out=o_ap, in_=i_ap)
                        inst.ins.queue = f"qPoolDynamic{((2 * g + q) % 4) or ''}"
                    else:
                        (nc.sync, nc.scalar)[q].dma_start(out=o_ap, in_=i_ap)


def _rewire_waits(nc):
    """Reorder Act-queue store triggers after all signs, then replace Tile's
    conservative / lane-aliased DMA waits with exact producer-based waits.

      xf[ch]     <- sign[ch - X_BUFS] (xf-slot WAR)
      w halves   <- (nothing; first on their HWDGE queues)
      sign[ch]   <- all 4 xf[ch] sub completions (RAW) + keep Tile's PE
                    wait (x8-slot WAR)
      copy[...]  <- keep Tile's PE wait only (psum RAW; osb pool has one
                    buffer per group, no WAR)
      store[g,q] <- last copy of group g (RAW)

    Waits are emitted as (producer's update-sem >= cumulative value after
    it); lane-order waits keep same-sem DMA updates ordered so >= waits
    cannot be satisfied by a later DMA that shares the semaphore.
    """
    import concourse.mybir as mybir

    # -- pass 0a: move Act-engine store DMAs after the last InstActivation --
    for f in nc.m.functions:
        for bb in f.blocks:
            ins_list = bb.instructions
            act_stores = [
                i
                for i in ins_list
                if type(i).__name__ == "InstDMACopy"
                and str(i.engine).endswith("Activation")
                and str(i.outs[0].memref).startswith("out")
            ]
            if not act_stores:
                continue
            rest = [i for i in ins_list if i not in act_stores]
            last_act = max(
                idx
                for idx, i in enumerate(rest)
                if type(i).__name__ == "InstActivation"
            )
            bb.instructions[:] = (
                rest[: last_act + 1] + act_stores + rest[last_act + 1 :]
            )

    # -- pass 0b: reposition Pool-queue store DMAs right after the load
    # trigger whose slot-WAR wait is looser than the store's copy-wait
    # (store[g] after xf[4g+10]), so they never head-of-line block a load --
    for f in nc.m.functions:
        for bb in f.blocks:
            ins_list = bb.instructions
            pool_stores = {}
            for i in ins_list:
                if (
                    type(i).__name__ == "InstDMACopy"
                    and str(i.engine).endswith("Pool")
                    and str(i.outs[0].memref).startswith("out")
                ):
                    g = int(i.outs[0].offset) // (512 * N)
                    pool_stores.setdefault(g, []).append(i)
            if not pool_stores:
                continue
            flat = [i for v in pool_stores.values() for i in v]
            rest = [i for i in ins_list if i not in flat]
            xf_pos = {}
            for idx, i in enumerate(rest):
                if type(i).__name__ == "InstDMACopy" and str(
                    i.outs[0].memref
                ).startswith("xf"):
                    ch = int(str(i.outs[0].memref)[2:].split("_")[0])
                    xf_pos[ch] = idx
            inserts = {}  # position -> [insts]
            for g, sts in pool_stores.items():
                tgt = xf_pos[min(4 * g + 10, N_CH - 1)]
                inserts.setdefault(tgt, []).extend(sts)
            new_list = []
            for idx, i in enumerate(rest):
                new_list.append(i)
                if idx in inserts:
                    new_list.extend(inserts[idx])
            bb.instructions[:] = new_list

    insts = []
    for f in nc.m.functions:
        for bb in f.blocks:
            insts.extend(bb.instructions)

    cum = {}
    upd_after = {}  # inst name -> (sem_name, sem_id, cum_value_after)
    lane_order = {}  # inst name -> SyncWait enforcing same-lane completion order
    xf_subs = {}  # ch -> [inst]
    signs = {}  # ch -> inst
    copies = {}  # g -> [inst]
    stores = {}  # g -> [inst]
    w_loads = []
    for ins in insts:
        si = getattr(ins, "sync_info", None)
        if si is None:
            continue
        for u in si.on_update or []:
            prev = cum.get(u.ant_name, 0)
            if prev > 0 and (
                u.ant_name.startswith("DMAHW") or u.ant_name.startswith("DMASW")
            ):
                lane_order[ins.name] = mybir.SyncWait(
                    sync_type="semaphore",
                    id=u.id,
                    ant_name=u.ant_name,
                    wait_mode="sem-ge-imm",
                    wait_value=prev,
                )
            cum[u.ant_name] = prev + u.update_value
            upd_after[ins.name] = (u.ant_name, u.id, cum[u.ant_name])
        memref = str(getattr(ins.outs[0], "memref", "")) if ins.outs else ""
        tn = type(ins).__name__
        if tn == "InstDMACopy" and memref.startswith("xf"):
            ch = int(memref[2 : memref.index("_")])
            xf_subs.setdefault(ch, []).append(ins)
        elif tn == "InstDMACopy" and memref.startswith("wT"):
            w_loads.append(ins)
        elif tn == "InstDMACopy" and memref.startswith("out"):
            off = int(ins.outs[0].offset)  # in f16 elements
            g = off // (512 * N)
            stores.setdefault(g, []).append(ins)
        elif tn == "InstActivation" and memref.startswith("x8"):
            ch = int(memref[2 : memref.index("_")])
            signs[ch] = ins
        elif tn == "InstTensorCopy" and memref.startswith("osb"):
            g = int(memref[3 : memref.index("_")])
            copies.setdefault(g, []).append(ins)

    assert sorted(xf_subs) == list(range(N_CH)) and all(
        len(v) == 1 for v in xf_subs.values()
    ), {k: len(v) for k, v in xf_subs.items()}
    assert sorted(signs) == list(range(N_CH))
    assert sorted(copies) == list(range(N_GRP)) and all(
        len(v) == 8 for v in copies.values()
    )
    assert sorted(stores) == list(range(N_GRP)) and all(
        len(v) == 2 for v in stores.values()
    )
    assert len(w_loads) == 2

    def wait_on(producer_ins):
        sem_name, sem_id, v = upd_after[producer_ins.name]
        return mybir.SyncWait(
            sync_type="semaphore",
            id=sem_id,
            ant_name=sem_name,
            wait_mode="sem-ge-imm",
            wait_value=v,
        )

    def keep_engine_waits(ins):
        return [
            w
            for w in (ins.sync_info.on_wait or [])
            if not (
                w.ant_name.startswith("DMAHW")
                or w.ant_name.startswith("DMASW")
                or w.ant_name.startswith("Activation")
                or w.ant_name.startswith("DVE")
            )
        ]

    def set_waits(ins, producers, extra=()):
        si = ins.sync_info
        waits = [wait_on(p) for p in producers if p is not None] + list(extra)
        lo = lane_order.get(ins.name)
        if lo is not None:
            waits.append(lo)
        ins.sync_info = mybir.SyncInfo(
            on_wait=waits, on_update=list(si.on_update or [])
        )

    for ch in range(N_CH):
        for ins in xf_subs[ch]:
            set_waits(ins, [signs[ch - X_BUFS]] if ch >= X_BUFS else [])
    for ins in w_loads:
        set_waits(ins, [])
    for ch in range(N_CH):
        set_waits(signs[ch], xf_subs[ch], extra=keep_engine_waits(signs[ch]))
    for g in range(N_GRP):
        for ins in copies[g]:
            set_waits(ins, [], extra=keep_engine_waits(ins))
        for ins in stores[g]:
            set_waits(ins, [copies[g][-1]])


def _legalize_dma_waits(nc):
    """Walrus caps in-struct sem waits (DMA_DIRECT2D takes 1, DMACopy 2).

    Tile's sem assignment is not transitively minimal and can emit 2-4 waits
    on DMA instructions. Hoist the excess into InstEventSemaphore wait-only
    instructions inserted just before the DMA on its triggering queue. This
    is sound: the queue executes the hoisted wait strictly before pushing the
    DMA descriptor, so the dependency is enforced (more conservatively) at
    trigger time instead of ring-pop time.
    """
    import concourse.mybir as mybir

    limits = {
        "InstDmaTransposeAnt": 1,
        "InstDMACopy": 1,
        "InstTensorCopy": 1,
        "InstActivation": 1,
        "InstMatmult": 1,
        "InstLdweights": 1,
        "InstMemset": 1,
        "InstTensorTensor": 1,
        "InstDrain": 1,
    }
    n_hoisted = 0
    for f in nc.m.functions:
        for bb in f.blocks:
            new_list = []
            for ins in bb.instructions:
                lim = limits.get(type(ins).__name__)
                si = getattr(ins, "sync_info", None)
                waits = list(si.on_wait) if si is not None and si.on_wait else []
                if lim is not None and len(waits) > lim:
                    # keep data-producer (engine-sem) waits in-struct first,
                    # then the freshest DMA-lane waits; hoist the rest
                    def keep_rank(w):
                        is_lane = w.ant_name.startswith(
                            "DMAHW"
                        ) or w.ant_name.startswith("DMASW")
                        return (1 if is_lane else 0, -w.wait_value)

                    waits_sorted = sorted(waits, key=keep_rank)
                    keep, hoist = waits_sorted[:lim], waits_sorted[lim:]
                    for ci in range(0, len(hoist), 2):
                        chunk = hoist[ci : ci + 2]
                        ev = mybir.InstEventSemaphore(
                            name=f"{ins.name}-prewait{ci // 2}",
                            engine=ins.engine,
                            ins=[],
                            outs=[],
                            sync_info=mybir.SyncInfo(on_wait=chunk, on_update=[]),
                        )
                        nc.inst_map[ev.name] = ev
                        new_list.append(ev)
                        n_hoisted += len(chunk)
                    ins.sync_info = mybir.SyncInfo(
                        on_wait=keep, on_update=list(si.on_update or [])
                    )
                new_list.append(ins)
            bb.instructions[:] = new_list
    return n_hoisted


def _build_nc():
    import concourse.bass as bass
    import concourse.mybir as mybir
    from concourse import tile

    nc = bass.Bass("TRN2", target_bir_lowering=False, num_swdge_queues=4)
    x_d = nc.dram_tensor(
        "x", [N_CH * P, 8 * MC], mybir.dt.float32, kind="ExternalInput"
    )
    w_d = nc.dram_tensor("W", [P, 8 * N], mybir.dt.float8e4, kind="ExternalInput")
    out_d = nc.dram_tensor(
        "out", [M_PER_CORE, N], mybir.dt.float16, kind="ExternalOutput"
    )
    with tile.TileContext(nc) as tc:
        build_binary_linear(tc, out_d.ap(), x_d.ap(), w_d.ap())
    _rewire_waits(nc)
    _legalize_dma_waits(nc)
    return nc


_cached = {}


def _get_nc():
    if "nc" not in _cached:
        _cached["nc"] = _build_nc()
    return _cached["nc"]


def kernel(x, W, _trace=False):
    from concourse import bass_utils

    import ml_dtypes

    xf = np.asarray(x, dtype=np.float32).reshape(M_TOTAL, K)
    # host re-layout (pure permutation): per core [ (g, b0, p), (j, c, u) ]
    # with m = 2048*core + 512g + 4u + b0 and i = 256j + 128c + p
    T = xf.reshape(N_CORES, 4, P, 4, 4, 2, P)  # (core, g, u, b0, j, c, p)
    xh = np.ascontiguousarray(T.transpose(0, 1, 3, 6, 4, 5, 2)).reshape(
        N_CORES, N_CH * P, 8 * MC
    )
    # pack sign(W) fp8: wq[p, (j, c, o)] = sign(W)[o, 256j + 128c + p]
    sT = np.sign(np.asarray(W, dtype=np.float32)).T.astype(ml_dtypes.float8_e4m3)
    wq = np.ascontiguousarray(
        sT.reshape(4, 2, P, N).transpose(2, 0, 1, 3)
    ).reshape(P, 8 * N)
    in_maps = [{"x": xh[i], "W": wq} for i in range(N_CORES)]
    nc = _get_nc()
    res = bass_utils.run_bass_kernel_spmd(
        nc, in_maps, core_ids=list(range(N_CORES)), trace=_trace
    )
    out = np.concatenate([r["out"] for r in res.results], axis=0)
    out = out.astype(np.float32).reshape(4, 4096, N)
    if _trace:
        kernel.last_results = res
    return out
